# revision 1
# baseline (speedup 1.0000x reference)
"""Distributed Trainium2 kernel for LoRA multi-head causal attention.

Problem: out = (softmax(causal(RoPE(x@wq.T + lora_q) @ RoPE(x@wk.T + lora_k)^T
               / sqrt(dh))) @ (x@wv.T + lora_v)) @ wo.T
Shapes: B=4, S=2048, D=2048, H=16 heads, DH=128, LoRA rank 16, fp32 I/O.

Sharding (8 NeuronCores): 2-D grid of (batch b = core//2) x (head-group
j = core%2, 8 heads each).  Each core computes its batch's attention for its
8 heads plus the partial o_proj for those heads' feature rows; partials are
summed with a pairwise ReduceScatter (cores 2b, 2b+1), each core emitting
half the sequence rows of batch b.  The host concatenates the halves.

Device-side plan per core (all matmuls bf16 with fp32 PSUM accumulation):
  A. cast x/weights to bf16; bounce weights through DRAM and read back
     transposed via the DMA xbar so the contraction dim lands on SBUF
     partitions; X^T is produced with PE transposes (PE is idle in A).
     wq/wk rows (and lq_up/lk_up rows) are loaded pair-deinterleaved
     (evens then odds within each head block) so the RoPE pair lanes
     become contiguous partition ranges.
  B. V = x@wv.T + lora_v in [s, m] layout, SBUF-resident; then per head
     Q^T/K^T [dh, s] with the LoRA rank-16 update accumulated into the
     same PSUM tile and a fused RoPE-and-cast eviction (1/sqrt(dh)
     pre-folded into Q's tables); Q^T/K^T bounce to DRAM.
  C. Attention, head outer / 512-query-chunk inner, causal blocks only:
     scores^T = K_tile @ Q^T (one transposed [128,128] mask tile on the
     diagonal), ACT exp straight out of PSUM into bf16 P^T, then P@V and
     the softmax denominator (ones-column matmul) accumulated over key
     tiles.  exp() needs no max-subtraction: scores are O(1) here by
     construction (std ~0.8, max ~4).
  D. o_proj from the attention outputs' transposed layout; the softmax
     normalization (a per-query scale) commutes with o_proj's row-linear
     form and is applied at the PSUM eviction via an f32r ones-broadcast
     matmul of 1/l.
  E. Pairwise ReduceScatter of the partial [2048, 2048] fp32 output.
"""

import sys

for _p in ("/opt/trn_rl_repo", "/root/.axon_site/_ro/trn_rl_repo"):
    if _p not in sys.path:
        sys.path.append(_p)

import numpy as np

import concourse.bass as bass
import concourse.mybir as mybir
import concourse.tile as tile
from concourse.masks import make_identity

F32 = mybir.dt.float32
F32R = mybir.dt.float32r
BF16 = mybir.dt.bfloat16
AF = mybir.ActivationFunctionType

B, S, D, H = 4, 2048, 2048, 16
DH = 128
R = 16
H_LOC = 8           # heads per core
M = H_LOC * DH      # 1024: local qkv feature dim
SCALE = 1.0 / float(np.sqrt(DH))
LORA_SCALE = 32.0 / 16.0
N_CORES = 8
SC = 512            # q/s chunk size
NQC = S // SC       # 4
NDT = D // 128      # 16 contraction tiles
NMT = M // 128      # 8 local m tiles (= local heads)
NST = S // 128      # 16 sequence tiles

# ---------------------------------------------------------------------------
# Walrus in this container rejects instructions carrying more than one sync
# wait ("Too many sync wait commands").  After Tile scheduling, move excess
# semaphore waits onto same-engine nop instructions inserted immediately
# before the over-subscribed instruction (same sequencer, same order, so
# semantics are unchanged).
# ---------------------------------------------------------------------------


def _split_sync_waits(nc, limit=1):
    for bb in nc.main_func.blocks:
        out = []
        changed = False
        for inst in bb.instructions:
            si = inst.sync_info
            waits = list(si.on_wait) if si is not None else []
            if len(waits) > limit:
                changed = True
                extra, keep = waits[:-limit], waits[-limit:]
                for i in range(0, len(extra), limit):
                    n = mybir.InstNoOp(name=nc.get_next_instruction_name())
                    n.engine = inst.engine
                    n.sync_info = mybir.SyncInfo(
                        on_wait=extra[i : i + limit], on_update=[]
                    )
                    nc.register_instruction(n)
                    out.append(n)
                inst.sync_info = mybir.SyncInfo(
                    on_wait=keep, on_update=list(si.on_update)
                )
            out.append(inst)
        if changed:
            bb.instructions = out


class FixedTileContext(tile.TileContext):
    def __exit__(self, exc_type, exc_value, traceback):
        result = super().__exit__(exc_type, exc_value, traceback)
        if exc_type is None:
            _split_sync_waits(self.nc)
        return result


# ---------------------------------------------------------------------------
# Kernel builder (one SPMD graph, identical on all 8 cores)
# ---------------------------------------------------------------------------


def build_nc():
    nc = bass.Bass(target_bir_lowering=False)

    x_ext = nc.declare_dram_parameter("x", [S, D], F32, isOutput=False)
    wq_ext = nc.declare_dram_parameter("wq", [M, D], F32, isOutput=False)
    wk_ext = nc.declare_dram_parameter("wk", [M, D], F32, isOutput=False)
    wv_ext = nc.declare_dram_parameter("wv", [M, D], F32, isOutput=False)
    wo_ext = nc.declare_dram_parameter("wo", [D, M], F32, isOutput=False)
    lqd_ext = nc.declare_dram_parameter("lqd", [R, D], F32, isOutput=False)
    lkd_ext = nc.declare_dram_parameter("lkd", [R, D], F32, isOutput=False)
    lvd_ext = nc.declare_dram_parameter("lvd", [R, D], F32, isOutput=False)
    lqu_ext = nc.declare_dram_parameter("lqu", [M, R], F32, isOutput=False)
    lku_ext = nc.declare_dram_parameter("lku", [M, R], F32, isOutput=False)
    lvu_ext = nc.declare_dram_parameter("lvu", [M, R], F32, isOutput=False)
    fcos_ext = nc.declare_dram_parameter("fcos", [S, DH // 2], F32, isOutput=False)
    fsin_ext = nc.declare_dram_parameter("fsin", [S, DH // 2], F32, isOutput=False)
    maskc_ext = nc.declare_dram_parameter("maskc", [128, 128], F32, isOutput=False)
    out_ext = nc.declare_dram_parameter("out", [S // 2, D], F32, isOutput=True)

    # DRAM internals.  Per-tile tensors where finer dependency granularity
    # buys pipeline overlap.
    wob = [nc.dram_tensor(f"wob{oc}", [SC, M], BF16) for oc in range(NQC)]
    qtd = [nc.dram_tensor(f"qtd{h}", [128, S], BF16) for h in range(H_LOC)]
    ktd = [nc.dram_tensor(f"ktd{h}", [128, S], BF16) for h in range(H_LOC)]
    out_partial = nc.dram_tensor("out_partial", [S, D], F32)
    out_rs = nc.dram_tensor("out_rs", [S // 2, D], F32)

    with FixedTileContext(nc) as tc:
        # ------------------------------------------------------------------
        # Persistent constants
        # ------------------------------------------------------------------
        with tc.tile_pool(name="const", bufs=1) as const:
            id_f32 = const.tile([128, 128], F32, tag="idf")
            make_identity(nc, id_f32)
            id_bf16 = const.tile([128, 128], BF16, tag="idh")
            make_identity(nc, id_bf16)
            ones_col = const.tile([128, 1], BF16, tag="ones_col")
            nc.vector.memset(ones_col[:, :], 1.0)
            ones_row_f = const.tile([1, 128], F32, tag="ones_row_f")
            nc.vector.memset(ones_row_f[:, :], 1.0)
            ones_row = const.tile([1, 128], F32R, tag="ones_row")
            nc.vector.tensor_copy(ones_row[:, :], ones_row_f[:, :])
            ones_row_h = const.tile([1, 128], BF16, tag="ones_row_h")
            nc.vector.tensor_copy(ones_row_h[:, :], ones_row_f[:, :])
            triT = const.tile([128, 128], F32, tag="triT")
            triT_h = const.tile([128, 128], BF16, tag="triT_h")
            ldsT = const.tile([128, 80 * NDT], BF16, tag="ldsT")
            nc.vector.memset(ldsT[:, :], 0.0)
            upTs = {k: const.tile([16, M], BF16, tag=f"upT{k}", name=f"upT{k}") for k in "qkv"}
            ldTs = {k: const.tile([16, S], BF16, tag=f"ldT{k}", name=f"ldT{k}") for k in "qkv"}

            _phases(
                nc, tc, locals(),
            )
    return nc


def _attn_epilogue(nc, g, sc_ps, linv_pool, at_pool, AT, qc, h, po, pl):
    BF16_ = BF16
    li = linv_pool.tile([1, SC], BF16_, tag="li", name=f"li{h}_{qc}")
    with nc.allow_low_precision(reason="1/l in bf16 for the broadcast matmul"):
        nc.vector.reciprocal(li[:, :], pl[:, :])
    pb = sc_ps.tile([128, SC], F32, tag="ps", name=f"pb{h}_{qc}")
    nc.tensor.matmul(
        pb[:, :], g["ones_row_h"][:, :], li[:, :], start=True, stop=True
    )
    lb = linv_pool.tile([128, SC], F32, tag="lb", name=f"lb{h}_{qc}")
    nc.vector.tensor_copy(lb[:, :], pb[:, :])
    at_ = at_pool.tile([128, SC], BF16_, tag=f"aT{h}", name=f"aT{h}_{qc}")
    nc.vector.tensor_mul(at_[:, :], po[:, :], lb[:, :])
    AT[h] = at_


def _phases(nc, tc, t):
    g = t
    x_ext, wo_ext = g["x_ext"], g["wo_ext"]
    fcos_ext, fsin_ext, maskc_ext = g["fcos_ext"], g["fsin_ext"], g["maskc_ext"]
    out_ext, out_partial, out_rs = g["out_ext"], g["out_partial"], g["out_rs"]
    wob = g["wob"]
    qtd, ktd = g["qtd"], g["ktd"]
    id_f32, id_bf16 = g["id_f32"], g["id_bf16"]
    ones_col, ones_row, triT = g["ones_col"], g["ones_row"], g["triT"]
    triT_h = g["triT_h"]
    ldsT, upTs, ldTs = g["ldsT"], g["upTs"], g["ldTs"]

    vs_ctx = tc.tile_pool(name="vs", bufs=1)
    vs_pool = vs_ctx.__enter__()
    VS = [vs_pool.tile([128, M], BF16, tag=f"v{st}", name=f"v{st}") for st in range(NST)]

    with tc.tile_pool(name="ab", bufs=1) as ab_pool:
        # rope tables live through phase B only.  Layout: the cos table has
        # cos duplicated on rows 0:64 and 64:128; same for sin.  This keeps
        # every rope product partition-aligned (GpSimd and SBUF-SBUF DVE
        # ops require equal base partitions).
        tabqC = ab_pool.tile([128, S], BF16, tag="tabqC")
        tabqS = ab_pool.tile([128, S], BF16, tag="tabqS")
        tabkC = ab_pool.tile([128, S], BF16, tag="tabkC")
        tabkS = ab_pool.tile([128, S], BF16, tag="tabkS")
        XT = [ab_pool.tile([128, S], BF16, tag=f"xT{dt}", name=f"xT{dt}") for dt in range(NDT)]

        with tc.tile_pool(name="stage", bufs=2) as stage:
            # --------------------------------------------------------------
            # Phase A1: x -> X^T, tables, lora constants, wv bounce.
            # Ordered so phase B1 (V projection) can start as early as
            # possible; wq/wk/wo casts are deferred past B1.
            # --------------------------------------------------------------
            with tc.tile_pool(name="stg_ps", bufs=1, space="PSUM") as stg_ps, \
                 tc.tile_pool(name="px_ps", bufs=2, space="PSUM") as px_ps:

                # X^T via PE transposes (bf16)
                for st in range(NST):
                    xh = stage.tile([128, D], BF16, tag="big_bf16")
                    nc.gpsimd.dma_start(
                        out=xh[:, :], in_=x_ext[st * 128 : (st + 1) * 128, :]
                    )
                    for dt in range(NDT):
                        px = px_ps.tile([128, 128], BF16, tag="px")
                        nc.tensor.transpose(
                            px[:, :], xh[:, dt * 128 : (dt + 1) * 128], id_bf16[:, :]
                        )
                        dst = XT[dt][:, st * 128 : (st + 1) * 128]
                        if st % 2 == 0:
                            nc.scalar.copy(dst, px[:, :])
                        else:
                            nc.vector.tensor_copy(dst, px[:, :])

                # mask corner -> transposed tri tile
                mk = stage.tile([128, 128], F32, tag="small_f32")
                nc.sync.dma_start(out=mk[:, :], in_=maskc_ext[:, :])
                pmk = stg_ps.tile([128, 128], F32, tag="pmk")
                nc.tensor.transpose(pmk[:, :], mk[:, :], id_f32[:, :])
                nc.vector.tensor_copy(triT[:, :], pmk[:, :])
                nc.vector.tensor_copy(triT_h[:, :], pmk[:, :])

                # rope tables (1/sqrt(dh) folded into the q tables)
                for st in range(NST):
                    fc = stage.tile([128, 64], F32, tag="fc")
                    fs = stage.tile([128, 64], F32, tag="fs")
                    nc.sync.dma_start(out=fc[:, :], in_=fcos_ext[st * 128 : (st + 1) * 128, :])
                    nc.sync.dma_start(out=fs[:, :], in_=fsin_ext[st * 128 : (st + 1) * 128, :])
                    pc = stg_ps.tile([64, 128], F32, tag="pc")
                    ps = stg_ps.tile([64, 128], F32, tag="ps")
                    nc.tensor.transpose(pc[:, :], fc[:, :], id_f32[:, :])
                    nc.tensor.transpose(ps[:, :], fs[:, :], id_f32[:, :])
                    sl = slice(st * 128, (st + 1) * 128)
                    nc.scalar.mul(tabqC[0:64, sl], pc[:, :], SCALE)
                    nc.scalar.mul(tabqC[64:128, sl], pc[:, :], SCALE)
                    nc.scalar.mul(tabqS[0:64, sl], ps[:, :], SCALE)
                    nc.scalar.mul(tabqS[64:128, sl], ps[:, :], SCALE)
                    nc.vector.tensor_copy(tabkC[0:64, sl], pc[:, :])
                    nc.vector.tensor_copy(tabkC[64:128, sl], pc[:, :])
                    nc.vector.tensor_copy(tabkS[0:64, sl], ps[:, :])
                    nc.vector.tensor_copy(tabkS[64:128, sl], ps[:, :])

                # lora downs: stacked [48, D] -> bf16 -> padded stationaries
                # (each 16-row group at a 32-aligned psum partition base)
                ldf = stage.tile([48, D], F32, tag="big_f32")
                nc.sync.dma_start(out=ldf[0:16, :], in_=g["lqd_ext"][:, :])
                nc.sync.dma_start(out=ldf[16:32, :], in_=g["lkd_ext"][:, :])
                nc.sync.dma_start(out=ldf[32:48, :], in_=g["lvd_ext"][:, :])
                ldh = stage.tile([48, D], BF16, tag="big_bf16")
                nc.vector.tensor_copy(ldh[:, :], ldf[:, :])
                for dt in range(NDT):
                    pt_ = stg_ps.tile([128, 48], BF16, tag="pldt")
                    nc.tensor.transpose(
                        pt_[:, :], ldh[:, dt * 128 : (dt + 1) * 128], id_bf16[0:48, 0:48]
                    )
                    base = dt * 80
                    nc.scalar.copy(ldsT[:, base : base + 16], pt_[:, 0:16])
                    nc.scalar.copy(ldsT[:, base + 32 : base + 48], pt_[:, 16:32])
                    nc.scalar.copy(ldsT[:, base + 64 : base + 80], pt_[:, 32:48])

                # lora ups: rows deinterleaved for q/k, natural for v
                for name, ext in (("q", g["lqu_ext"]), ("k", g["lku_ext"]), ("v", g["lvu_ext"])):
                    rr = ext.rearrange("(mt q p) r -> p mt q r", mt=NMT, q=64, p=2)
                    for mt in range(NMT):
                        uf = stage.tile([128, R], F32, tag="uf")
                        if name in ("q", "k"):
                            nc.sync.dma_start(out=uf[0:64, :], in_=rr[0, mt])
                            nc.sync.dma_start(out=uf[64:128, :], in_=rr[1, mt])
                        else:
                            nc.sync.dma_start(
                                out=uf[:, :], in_=ext[mt * 128 : (mt + 1) * 128, :]
                            )
                        uh = stage.tile([128, R], BF16, tag="uh")
                        nc.vector.tensor_copy(uh[:, :], uf[:, :])
                        pu = stg_ps.tile([R, 128], BF16, tag="pu")
                        nc.tensor.transpose(pu[:, :], uh[:, :], id_bf16[:, :])
                        nc.scalar.copy(upTs[name][:, mt * 128 : (mt + 1) * 128], pu[:, :])


            # --------------------------------------------------------------
            # Phase B1: lora mids + V projection (SBUF-resident V)
            # --------------------------------------------------------------
            with tc.tile_pool(name="wtv", bufs=16) as wtv_pool, \
                 tc.tile_pool(name="proj_ps1", bufs=2, space="PSUM") as proj_ps1, \
                 tc.tile_pool(name="tw_ps", bufs=2, space="PSUM") as tw_ps, \
                 tc.tile_pool(name="ld_ps", bufs=1, space="PSUM") as ld_ps:
                for qc in range(NQC):
                    pld = ld_ps.tile([80, SC], F32, tag="pld")
                    for dt in range(NDT):
                        nc.tensor.matmul(
                            pld[:, :],
                            ldsT[:, dt * 80 : (dt + 1) * 80],
                            XT[dt][:, qc * SC : (qc + 1) * SC],
                            start=(dt == 0),
                            stop=(dt == NDT - 1),
                        )
                    sl = slice(qc * SC, (qc + 1) * SC)
                    nc.scalar.mul(ldTs["q"][:, sl], pld[0:16, :], LORA_SCALE)
                    nc.scalar.mul(ldTs["k"][:, sl], pld[32:48, :], LORA_SCALE)
                    nc.scalar.mul(ldTs["v"][:, sl], pld[64:80, :], LORA_SCALE)

                wts = [
                    wtv_pool.tile([128, M], BF16, tag="wtv", name=f"wvT{dt}")
                    for dt in range(NDT)
                ]
                for mt in range(NMT):
                    wh = stage.tile([128, D], BF16, tag="big_bf16")
                    nc.gpsimd.dma_start(
                        out=wh[:, :], in_=g["wv_ext"][mt * 128 : (mt + 1) * 128, :]
                    )
                    for dt in range(NDT):
                        ptw = tw_ps.tile([128, 128], BF16, tag="ptw")
                        nc.tensor.transpose(
                            ptw[:, :], wh[:, dt * 128 : (dt + 1) * 128], id_bf16[:, :]
                        )
                        dst = wts[dt][:, mt * 128 : (mt + 1) * 128]
                        if dt % 2 == 0:
                            nc.scalar.copy(dst, ptw[:, :])
                        else:
                            nc.vector.tensor_copy(dst, ptw[:, :])
                for st in range(NST):
                    pv = proj_ps1.tile([128, M], F32, tag="pv")
                    ssl = slice(st * 128, (st + 1) * 128)
                    for sub in range(2):
                        psl = slice(sub * SC, (sub + 1) * SC)
                        for dt in range(NDT):
                            nc.tensor.matmul(
                                pv[:, psl], XT[dt][:, ssl], wts[dt][:, psl],
                                start=(dt == 0), stop=False,
                            )
                        nc.tensor.matmul(
                            pv[:, psl], ldTs["v"][:, ssl], upTs["v"][:, psl],
                            start=False, stop=True,
                        )
                    nc.vector.tensor_copy(VS[st][:, :], pv[:, :])

            # --------------------------------------------------------------
            # Phase A2: wq/wk/wo casts (no PSUM; overlaps B1 via scheduler)
            # --------------------------------------------------------------
            for oc in range(NQC):
                for half in range(4):
                    wh = stage.tile([128, M], BF16, tag="big_bf16")
                    r0 = oc * SC + half * 128
                    nc.gpsimd.dma_start(out=wh[:, 0:M], in_=wo_ext[r0 : r0 + 128, :])
                    nc.sync.dma_start(
                        out=wob[oc][half * 128 : (half + 1) * 128, :], in_=wh[:, 0:M]
                    )

            # --------------------------------------------------------------
            # Phase B2: Q^T / K^T, head outer (q then k per head so that
            # attention on head h can begin while head h+1 projects), with
            # the rope eviction split across ACT (psum spill), DVE, and
            # GpSimd (2 of the 4 products).
            # --------------------------------------------------------------
            HS = S // 2  # 1024: half the sequence per psum tile
            with tc.tile_pool(name="wtt", bufs=20) as wtt_pool, \
                 tc.tile_pool(name="proj_ps2", bufs=2, space="PSUM") as proj_ps2, \
                 tc.tile_pool(name="tw2_ps", bufs=2, space="PSUM") as tw2_ps, \
                 tc.tile_pool(name="rope_tmp", bufs=3) as rope_tmp, \
                 tc.tile_pool(name="qk_ev", bufs=6) as qk_ev:
                rr_q = g["wq_ext"].rearrange("(mt q p) d -> p mt q d", mt=NMT, q=64, p=2)
                rr_k = g["wk_ext"].rearrange("(mt q p) d -> p mt q d", mt=NMT, q=64, p=2)
                for mt in range(NMT):
                    for which, rr, outd, tabC, tabS in (
                        ("q", rr_q, qtd, tabqC, tabqS),
                        ("k", rr_k, ktd, tabkC, tabkS),
                    ):
                        wh = stage.tile([128, D], BF16, tag="big_bf16")
                        nc.gpsimd.dma_start(out=wh[0:64, :], in_=rr[0, mt])
                        nc.gpsimd.dma_start(out=wh[64:128, :], in_=rr[1, mt])
                        wts = []
                        for dt in range(NDT):
                            ptw = tw2_ps.tile([128, 128], BF16, tag="ptw2")
                            nc.tensor.transpose(
                                ptw[:, :], wh[:, dt * 128 : (dt + 1) * 128], id_bf16[:, :]
                            )
                            wtt = wtt_pool.tile([128, 128], BF16, tag="wtt")
                            nc.scalar.copy(wtt[:, :], ptw[:, :])
                            wts.append(wtt)
                        for half in range(2):
                            sl = slice(half * HS, (half + 1) * HS)
                            pq = proj_ps2.tile([128, HS], F32, tag="pq")
                            for sub in range(2):
                                psl = slice(sub * SC, (sub + 1) * SC)
                                xsl = slice(half * HS + sub * SC, half * HS + (sub + 1) * SC)
                                for dt in range(NDT):
                                    nc.tensor.matmul(
                                        pq[:, psl], wts[dt][:, :], XT[dt][:, xsl],
                                        start=(dt == 0), stop=False,
                                    )
                                nc.tensor.matmul(
                                    pq[:, psl],
                                    upTs[which][:, mt * 128 : (mt + 1) * 128],
                                    ldTs[which][:, xsl],
                                    start=False, stop=True,
                                )
                            # rope on deinterleaved pairs: rows 0:64 = a
                            # (even lanes), 64:128 = b (odd lanes); tab rows
                            # 0:64 cos, 64:128 sin.
                            if mt < 6:
                                qsb = rope_tmp.tile([128, HS], BF16, tag="qsb")
                                nc.scalar.copy(qsb[:, :], pq[:, :])
                            t1 = rope_tmp.tile([128, HS], BF16, tag="t1")
                            t2 = rope_tmp.tile([128, HS], BF16, tag="t2")
                            # one full-width product, fully aligned:
                            # t1 rows 0:64 = a*cos, rows 64:128 = b*cos.
                            # GpSimd helps mid-phase; near the phase tail its
                            # queue backlog would delay the final evictions
                            # (and the PSUM release attention waits on), so
                            # the last heads use DVE instead.
                            if mt < 6:
                                nc.gpsimd.tensor_mul(t1[:, :], qsb[:, :], tabC[:, sl])
                            else:
                                nc.vector.tensor_mul(t1[:, :], pq[:, :], tabC[:, sl])
                            # cross-half products from psum on DVE (the psum
                            # operand may change base partition):
                            # t2 rows 0:64 = b*sin, rows 64:128 = a*sin
                            nc.vector.tensor_mul(t2[0:64, :], pq[64:128, :], tabS[0:64, sl])
                            nc.vector.tensor_mul(t2[64:128, :], pq[0:64, :], tabS[64:128, sl])
                            ev = qk_ev.tile([128, HS], BF16, tag="ev")
                            # even out = a*cos - b*sin ; odd out = a*sin + b*cos
                            nc.vector.tensor_sub(ev[0:64, :], t1[0:64, :], t2[0:64, :])
                            nc.vector.tensor_add(ev[64:128, :], t2[64:128, :], t1[64:128, :])
                            nc.sync.dma_start(out=outd[mt][:, sl], in_=ev[:, :])

    # ----------------------------------------------------------------------
    # Phases C+D interleaved, query-chunk outer: attention for all heads of
    # one 512-query chunk, then that chunk's o_proj rows and its pairwise
    # ReduceScatter — the collective for chunk c overlaps attention of
    # chunk c+1.
    # ----------------------------------------------------------------------
    with tc.tile_pool(name="at", bufs=2) as at_pool, \
         tc.tile_pool(name="wo_t", bufs=32) as wo_pool, \
         tc.tile_pool(name="qk_ld", bufs=6) as qk_ld, \
         tc.tile_pool(name="pt", bufs=8) as pt_pool, \
         tc.tile_pool(name="o_sb", bufs=4) as o_sb, \
         tc.tile_pool(name="sc_ps", bufs=2, space="PSUM") as sc_ps, \
         tc.tile_pool(name="pv_ps", bufs=2, space="PSUM") as pv_ps, \
         tc.tile_pool(name="l_ps", bufs=2, space="PSUM") as l_ps, \
         tc.tile_pool(name="o_ps", bufs=2, space="PSUM") as o_ps, \
         tc.tile_pool(name="linv", bufs=4) as linv_pool:

        wts_o = {}

        pre = {}
        for qc in range(NQC):
            nk = 4 * (qc + 1)
            AT = {}
            pending = []
            for h in range(H_LOC):
                if h in pre:
                    kth, qth = pre.pop(h)
                else:
                    kth = qk_ld.tile([128, S], BF16, tag="kth")
                    nc.sync.dma_start(
                        out=kth[:, 0 : nk * 128], in_=g["ktd"][h][:, 0 : nk * 128]
                    )
                    qth = qk_ld.tile([128, SC], BF16, tag="qth")
                    nc.sync.dma_start(
                        out=qth[:, :], in_=g["qtd"][h][:, qc * SC : (qc + 1) * SC]
                    )
                po = pv_ps.tile([128, SC], F32, tag="po")
                pl = l_ps.tile([1, SC], F32, tag="pl")
                for ki in range(nk):
                    q_off = max(0, ki * 128 - qc * SC)
                    csl = slice(q_off, SC)
                    ps_ = sc_ps.tile([128, SC], F32, tag="ps")
                    diag = ki * 128 >= qc * SC
                    nc.tensor.matmul(
                        ps_[:, csl],
                        kth[:, ki * 128 : (ki + 1) * 128],
                        qth[:, csl],
                        start=True, stop=not diag,
                    )
                    if diag:  # causal mask accumulated on the PE itself
                        nc.tensor.matmul(
                            ps_[:, q_off : q_off + 128],
                            id_bf16[:, :],
                            triT_h[:, :],
                            start=False, stop=True,
                        )
                    pt_ = pt_pool.tile([128, SC], BF16, tag="pt")
                    nc.scalar.activation(pt_[:, csl], ps_[:, csl], AF.Exp)
                    nc.tensor.matmul(
                        pl[:, csl], ones_col[:, :], pt_[:, csl],
                        start=(ki == 0), stop=(ki == nk - 1),
                    )
                    nc.tensor.matmul(
                        po[:, csl],
                        VS[ki][:, h * 128 : (h + 1) * 128],
                        pt_[:, csl],
                        start=(ki == 0), stop=(ki == nk - 1),
                    )
                # head epilogue is emitted one head late (see below) so the
                # broadcast matmul never stalls the in-order PE queue on the
                # reciprocal.
                pending.append((h, po, pl))
                if len(pending) > 1:
                    _attn_epilogue(nc, g, sc_ps, linv_pool, at_pool, AT, qc, *pending.pop(0))

            while pending:
                _attn_epilogue(nc, g, sc_ps, linv_pool, at_pool, AT, qc, *pending.pop(0))

            if qc == 0:
                # o_proj weights load here -- after chunk 0's K/Q streams,
                # before they are first consumed (their 32 slow xbar
                # transposes would otherwise delay attention's first loads)
                for oc in range(NQC):
                    for mt in range(NMT):
                        wtt = wo_pool.tile([128, SC], BF16, tag="wot")
                        nc.sync.dma_start_transpose(
                            out=wtt[:, :], in_=wob[oc][:, mt * 128 : (mt + 1) * 128]
                        )
                        wts_o[(oc, mt)] = wtt

            # prefetch the next chunk's first heads before o_proj floods the
            # DMA queues with output traffic
            if qc < NQC - 1:
                nk2 = 4 * (qc + 2)
                # later chunks stream more K data and face more RS traffic:
                # prefetch deeper for them
                for h2 in range(2 + qc):
                    kth2 = qk_ld.tile([128, S], BF16, tag="kth", name=f"kpre{qc}_{h2}")
                    nc.sync.dma_start(
                        out=kth2[:, 0 : nk2 * 128], in_=g["ktd"][h2][:, 0 : nk2 * 128]
                    )
                    qth2 = qk_ld.tile([128, SC], BF16, tag="qth", name=f"qpre{qc}_{h2}")
                    nc.sync.dma_start(
                        out=qth2[:, :],
                        in_=g["qtd"][h2][:, (qc + 1) * SC : (qc + 2) * SC],
                    )
                    pre[h2] = (kth2, qth2)

            # o_proj rows of this chunk + its ReduceScatter
            for st in range(qc * 4, (qc + 1) * 4):
                for oc in range(NQC):
                    posum = o_ps.tile([128, SC], F32, tag="posum")
                    for mt in range(NMT):
                        nc.tensor.matmul(
                            posum[:, :],
                            AT[mt][:, (st % 4) * 128 : (st % 4 + 1) * 128],
                            wts_o[(oc, mt)][:, :],
                            start=(mt == 0), stop=(mt == NMT - 1),
                        )
                    ot = o_sb.tile([128, SC], F32, tag="ot")
                    nc.vector.tensor_copy(ot[:, :], posum[:, :])
                    nc.sync.dma_start(
                        out=out_partial[
                            st * 128 : (st + 1) * 128, oc * SC : (oc + 1) * SC
                        ],
                        in_=ot[:, :],
                    )
            if qc < NQC - 1:
                parts = [(qc * SC, (qc + 1) * SC)]
            else:  # halve the last chunk so less of its collective is exposed
                parts = [(qc * SC, qc * SC + 256), (qc * SC + 256, (qc + 1) * SC)]
            for r0, r1 in parts:
                nc.gpsimd.collective_compute(
                    "ReduceScatter",
                    mybir.AluOpType.add,
                    replica_groups=[[0, 1], [2, 3], [4, 5], [6, 7]],
                    ins=[out_partial[r0:r1, :].opt()],
                    outs=[out_rs[r0 // 2 : r1 // 2, :].opt()],
                )
                nc.sync.dma_start(
                    out=out_ext[r0 // 2 : r1 // 2, :],
                    in_=out_rs[r0 // 2 : r1 // 2, :],
                )

    vs_ctx.__exit__(None, None, None)

# ---------------------------------------------------------------------------
# Host entry point
# ---------------------------------------------------------------------------

_NC_CACHE = None


def _get_nc():
    global _NC_CACHE
    if _NC_CACHE is None:
        _NC_CACHE = build_nc()
    return _NC_CACHE


def kernel(
    x, wq, wk, wv, wo,
    lq_down, lq_up, lk_down, lk_up, lv_down, lv_up,
    freqs_cos, freqs_sin, mask,
):
    """Full inputs in, full [B, S, D] output out; 8-core SPMD inside."""
    from concourse.bass_utils import run_bass_kernel_spmd

    x = np.asarray(x, dtype=np.float32)
    maskc = np.ascontiguousarray(np.asarray(mask, dtype=np.float32)[:128, :128])
    fcos = np.ascontiguousarray(np.asarray(freqs_cos, dtype=np.float32))
    fsin = np.ascontiguousarray(np.asarray(freqs_sin, dtype=np.float32))

    in_maps = []
    for c in range(N_CORES):
        b, j = c // 2, c % 2
        msl = slice(j * M, (j + 1) * M)
        in_maps.append({
            "x": np.ascontiguousarray(x[b]),
            "wq": np.ascontiguousarray(np.asarray(wq, np.float32)[msl, :]),
            "wk": np.ascontiguousarray(np.asarray(wk, np.float32)[msl, :]),
            "wv": np.ascontiguousarray(np.asarray(wv, np.float32)[msl, :]),
            "wo": np.ascontiguousarray(np.asarray(wo, np.float32)[:, msl]),
            "lqd": np.ascontiguousarray(np.asarray(lq_down, np.float32)),
            "lkd": np.ascontiguousarray(np.asarray(lk_down, np.float32)),
            "lvd": np.ascontiguousarray(np.asarray(lv_down, np.float32)),
            "lqu": np.ascontiguousarray(np.asarray(lq_up, np.float32)[msl, :]),
            "lku": np.ascontiguousarray(np.asarray(lk_up, np.float32)[msl, :]),
            "lvu": np.ascontiguousarray(np.asarray(lv_up, np.float32)[msl, :]),
            "fcos": fcos,
            "fsin": fsin,
            "maskc": maskc,
        })

    nc = _get_nc()
    res = run_bass_kernel_spmd(nc, in_maps, list(range(N_CORES)), trace=False)

    return assemble(res.results)


def assemble(results):
    # Chunked pairwise ReduceScatter: core (2b+j) chunk c holds the reduced
    # global rows [c*512 + j*256, c*512 + j*256 + 256) of batch b at local
    # rows [c*256, (c+1)*256).
    out = np.empty((B, S, D), dtype=np.float32)
    chunks = [(c * SC, (c + 1) * SC) for c in range(NQC - 1)]
    chunks += [((NQC - 1) * SC, (NQC - 1) * SC + 256), ((NQC - 1) * SC + 256, S)]
    for b in range(B):
        for j in range(2):
            buf = results[2 * b + j]["out"]
            for r0, r1 in chunks:
                n = (r1 - r0) // 2
                out[b, r0 + j * n : r0 + (j + 1) * n, :] = buf[r0 // 2 : r0 // 2 + n, :]
    return out



# revision 17
# speedup vs baseline: 1.0028x; 1.0028x over previous
"""Distributed Trainium2 kernel for LoRA multi-head causal attention.

Problem: out = (softmax(causal(RoPE(x@wq.T + lora_q) @ RoPE(x@wk.T + lora_k)^T
               / sqrt(dh))) @ (x@wv.T + lora_v)) @ wo.T
Shapes: B=4, S=2048, D=2048, H=16 heads, DH=128, LoRA rank 16, fp32 I/O.

Sharding (8 NeuronCores): 2-D grid of (batch b = core//2) x (head-group
j = core%2, 8 heads each).  Each core computes its batch's attention for its
8 heads plus the partial o_proj for those heads' feature rows; partials are
summed with a pairwise ReduceScatter (cores 2b, 2b+1), each core emitting
half the sequence rows of batch b.  The host concatenates the halves.

Device-side plan per core (all matmuls bf16 with fp32 PSUM accumulation):
  A. cast x/weights to bf16; bounce weights through DRAM and read back
     transposed via the DMA xbar so the contraction dim lands on SBUF
     partitions; X^T is produced with PE transposes (PE is idle in A).
     wq/wk rows (and lq_up/lk_up rows) are loaded pair-deinterleaved
     (evens then odds within each head block) so the RoPE pair lanes
     become contiguous partition ranges.
  B. V = x@wv.T + lora_v in [s, m] layout, SBUF-resident; then per head
     Q^T/K^T [dh, s] with the LoRA rank-16 update accumulated into the
     same PSUM tile and a fused RoPE-and-cast eviction (1/sqrt(dh)
     pre-folded into Q's tables); Q^T/K^T bounce to DRAM.
  C. Attention, head outer / 512-query-chunk inner, causal blocks only:
     scores^T = K_tile @ Q^T (one transposed [128,128] mask tile on the
     diagonal), ACT exp straight out of PSUM into bf16 P^T, then P@V and
     the softmax denominator (ones-column matmul) accumulated over key
     tiles.  exp() needs no max-subtraction: scores are O(1) here by
     construction (std ~0.8, max ~4).
  D. o_proj from the attention outputs' transposed layout; the per-head
     1/l normalization is applied to the attention output at its PSUM
     eviction via a ones-broadcast matmul of 1/l.
  E. Pairwise ReduceScatter of the partial fp32 output, issued per
     128-row st tile so collectives overlap the next tile's o_proj.
"""

import sys

for _p in ("/opt/trn_rl_repo", "/root/.axon_site/_ro/trn_rl_repo"):
    if _p not in sys.path:
        sys.path.append(_p)

import numpy as np

import concourse.bass as bass
import concourse.bass_isa as bass_isa
import concourse.mybir as mybir
import concourse.tile as tile
from concourse.masks import make_identity

F32 = mybir.dt.float32
F32R = mybir.dt.float32r
BF16 = mybir.dt.bfloat16
AF = mybir.ActivationFunctionType

B, S, D, H = 4, 2048, 2048, 16
DH = 128
R = 16
H_LOC = 8           # heads per core
M = H_LOC * DH      # 1024: local qkv feature dim
SCALE = 1.0 / float(np.sqrt(DH))
LORA_SCALE = 32.0 / 16.0
N_CORES = 8
SC = 512            # q/s chunk size
NQC = S // SC       # 4
NDT = D // 128      # 16 contraction tiles
NMT = M // 128      # 8 local m tiles (= local heads)
NST = S // 128      # 16 sequence tiles

# ---------------------------------------------------------------------------
# Walrus in this container rejects instructions carrying more than one sync
# wait ("Too many sync wait commands").  After Tile scheduling, move excess
# semaphore waits onto same-engine nop instructions inserted immediately
# before the over-subscribed instruction (same sequencer, same order, so
# semantics are unchanged).
# ---------------------------------------------------------------------------


def _split_sync_waits(nc, limit=1):
    for bb in nc.main_func.blocks:
        out = []
        changed = False
        for inst in bb.instructions:
            si = inst.sync_info
            waits = list(si.on_wait) if si is not None else []
            if len(waits) > limit:
                changed = True
                extra, keep = waits[:-limit], waits[-limit:]
                for i in range(0, len(extra), limit):
                    n = mybir.InstNoOp(name=nc.get_next_instruction_name())
                    n.engine = inst.engine
                    n.sync_info = mybir.SyncInfo(
                        on_wait=extra[i : i + limit], on_update=[]
                    )
                    nc.register_instruction(n)
                    out.append(n)
                inst.sync_info = mybir.SyncInfo(
                    on_wait=keep, on_update=list(si.on_update)
                )
            out.append(inst)
        if changed:
            bb.instructions = out


class FixedTileContext(tile.TileContext):
    def __exit__(self, exc_type, exc_value, traceback):
        result = super().__exit__(exc_type, exc_value, traceback)
        if exc_type is None:
            _split_sync_waits(self.nc)
        return result


# ---------------------------------------------------------------------------
# Kernel builder (one SPMD graph, identical on all 8 cores)
# ---------------------------------------------------------------------------


def build_nc():
    nc = bass.Bass(target_bir_lowering=False)

    x_ext = nc.declare_dram_parameter("x", [S, D], F32, isOutput=False)
    wq_ext = nc.declare_dram_parameter("wq", [M, D], F32, isOutput=False)
    wk_ext = nc.declare_dram_parameter("wk", [M, D], F32, isOutput=False)
    wv_ext = nc.declare_dram_parameter("wv", [M, D], F32, isOutput=False)
    wo_ext = nc.declare_dram_parameter("wo", [D, M], F32, isOutput=False)
    lqd_ext = nc.declare_dram_parameter("lqd", [R, D], F32, isOutput=False)
    lkd_ext = nc.declare_dram_parameter("lkd", [R, D], F32, isOutput=False)
    lvd_ext = nc.declare_dram_parameter("lvd", [R, D], F32, isOutput=False)
    lqu_ext = nc.declare_dram_parameter("lqu", [M, R], F32, isOutput=False)
    lku_ext = nc.declare_dram_parameter("lku", [M, R], F32, isOutput=False)
    lvu_ext = nc.declare_dram_parameter("lvu", [M, R], F32, isOutput=False)
    fcos_ext = nc.declare_dram_parameter("fcos", [S, DH // 2], F32, isOutput=False)
    fsin_ext = nc.declare_dram_parameter("fsin", [S, DH // 2], F32, isOutput=False)
    maskc_ext = nc.declare_dram_parameter("maskc", [128, 128], F32, isOutput=False)
    out_ext = nc.declare_dram_parameter("out", [S // 2, D], F32, isOutput=True)

    # DRAM internals.  Per-tile tensors where finer dependency granularity
    # buys pipeline overlap.
    wob = [nc.dram_tensor(f"wob{oc}", [SC, M], BF16) for oc in range(NQC)]
    qtd = [nc.dram_tensor(f"qtd{h}", [128, S], BF16) for h in range(H_LOC)]
    ktd = [nc.dram_tensor(f"ktd{h}", [128, S], BF16) for h in range(H_LOC)]
    out_partial = nc.dram_tensor("out_partial", [S, D], F32)
    out_rs = nc.dram_tensor("out_rs", [S // 2, D], F32)

    with FixedTileContext(nc) as tc:
        # ------------------------------------------------------------------
        # Persistent constants
        # ------------------------------------------------------------------
        with tc.tile_pool(name="const", bufs=1) as const:
            id_f32 = const.tile([128, 128], F32, tag="idf")
            make_identity(nc, id_f32)
            id_bf16 = const.tile([128, 128], BF16, tag="idh")
            make_identity(nc, id_bf16)
            ones_col = const.tile([128, 1], BF16, tag="ones_col")
            nc.vector.memset(ones_col[:, :], 1.0)
            ones_row_f = const.tile([1, 128], F32, tag="ones_row_f")
            nc.vector.memset(ones_row_f[:, :], 1.0)
            ones_row = const.tile([1, 128], F32R, tag="ones_row")
            nc.vector.tensor_copy(ones_row[:, :], ones_row_f[:, :])
            ones_row_h = const.tile([1, 128], BF16, tag="ones_row_h")
            nc.vector.tensor_copy(ones_row_h[:, :], ones_row_f[:, :])
            triT = const.tile([128, 128], F32, tag="triT")
            triT_h = const.tile([128, 128], BF16, tag="triT_h")
            ldsT = const.tile([128, 80 * NDT], BF16, tag="ldsT")
            nc.vector.memset(ldsT[:, :], 0.0)
            upTs = {k: const.tile([16, M], BF16, tag=f"upT{k}", name=f"upT{k}") for k in "qkv"}
            ldTs = {k: const.tile([16, S], BF16, tag=f"ldT{k}", name=f"ldT{k}") for k in "qkv"}

            _phases(
                nc, tc, locals(),
            )
    return nc


def _attn_epilogue(nc, g, sc_ps, linv_pool, at_pool, AT, qc, h, po, pl):
    BF16_ = BF16
    li = linv_pool.tile([1, SC], BF16_, tag="li", name=f"li{h}_{qc}")
    with nc.allow_low_precision(reason="1/l in bf16 for the broadcast matmul"):
        nc.vector.reciprocal(li[:, :], pl[:, :])
    pb = sc_ps.tile([128, SC], F32, tag="ps", name=f"pb{h}_{qc}")
    nc.tensor.matmul(
        pb[:, :], g["ones_row_h"][:, :], li[:, :], start=True, stop=True
    )
    lb = linv_pool.tile([128, SC], F32, tag="lb", name=f"lb{h}_{qc}")
    nc.vector.tensor_copy(lb[:, :], pb[:, :])
    at_ = at_pool.tile([128, SC], BF16_, tag=f"aT{h}", name=f"aT{h}_{qc}")
    nc.vector.tensor_mul(at_[:, :], po[:, :], lb[:, :])
    AT[h] = at_


def _phases(nc, tc, t):
    g = t
    x_ext, wo_ext = g["x_ext"], g["wo_ext"]
    fcos_ext, fsin_ext, maskc_ext = g["fcos_ext"], g["fsin_ext"], g["maskc_ext"]
    out_ext, out_partial, out_rs = g["out_ext"], g["out_partial"], g["out_rs"]
    wob = g["wob"]
    qtd, ktd = g["qtd"], g["ktd"]
    id_f32, id_bf16 = g["id_f32"], g["id_bf16"]
    ones_col, ones_row, triT = g["ones_col"], g["ones_row"], g["triT"]
    triT_h = g["triT_h"]
    ldsT, upTs, ldTs = g["ldsT"], g["upTs"], g["ldTs"]

    vs_ctx = tc.tile_pool(name="vs", bufs=1)
    vs_pool = vs_ctx.__enter__()
    VS = [vs_pool.tile([128, M], BF16, tag=f"v{st}", name=f"v{st}") for st in range(NST)]

    with tc.tile_pool(name="ab", bufs=1) as ab_pool:
        # rope tables live through phase B only.  Layout: the cos table has
        # cos duplicated on rows 0:64 and 64:128; same for sin.  This keeps
        # every rope product partition-aligned (GpSimd and SBUF-SBUF DVE
        # ops require equal base partitions).
        tabqC = ab_pool.tile([128, S], BF16, tag="tabqC")
        tabqS = ab_pool.tile([128, S], BF16, tag="tabqS")
        tabkC = ab_pool.tile([128, S], BF16, tag="tabkC")
        tabkS = ab_pool.tile([128, S], BF16, tag="tabkS")
        XT = [ab_pool.tile([128, S], BF16, tag=f"xT{dt}", name=f"xT{dt}") for dt in range(NDT)]

        with tc.tile_pool(name="stage", bufs=2) as stage:
            # --------------------------------------------------------------
            # Phase A1: x -> X^T, tables, lora constants, wv bounce.
            # Ordered so phase B1 (V projection) can start as early as
            # possible; wq/wk/wo casts are deferred past B1.
            # --------------------------------------------------------------
            with tc.tile_pool(name="stg_ps", bufs=1, space="PSUM") as stg_ps, \
                 tc.tile_pool(name="px_ps", bufs=2, space="PSUM") as px_ps:

                # X^T via PE transposes (bf16)
                for st in range(NST):
                    xh = stage.tile([128, D], BF16, tag="big_bf16")
                    nc.gpsimd.dma_start(
                        out=xh[:, :], in_=x_ext[st * 128 : (st + 1) * 128, :]
                    )
                    for dt in range(NDT):
                        px = px_ps.tile([128, 128], BF16, tag="px")
                        nc.tensor.transpose(
                            px[:, :], xh[:, dt * 128 : (dt + 1) * 128], id_bf16[:, :]
                        )
                        dst = XT[dt][:, st * 128 : (st + 1) * 128]
                        if st % 2 == 0:
                            nc.scalar.copy(dst, px[:, :])
                        else:
                            nc.vector.tensor_copy(dst, px[:, :])

                # mask corner -> transposed tri tile
                mk = stage.tile([128, 128], F32, tag="small_f32")
                nc.sync.dma_start(out=mk[:, :], in_=maskc_ext[:, :])
                pmk = stg_ps.tile([128, 128], F32, tag="pmk")
                nc.tensor.transpose(pmk[:, :], mk[:, :], id_f32[:, :])
                nc.vector.tensor_copy(triT[:, :], pmk[:, :])
                nc.vector.tensor_copy(triT_h[:, :], pmk[:, :])

                # rope tables (1/sqrt(dh) folded into the q tables)
                for st in range(NST):
                    fc = stage.tile([128, 64], F32, tag="fc")
                    fs = stage.tile([128, 64], F32, tag="fs")
                    nc.sync.dma_start(out=fc[:, :], in_=fcos_ext[st * 128 : (st + 1) * 128, :])
                    nc.sync.dma_start(out=fs[:, :], in_=fsin_ext[st * 128 : (st + 1) * 128, :])
                    pc = stg_ps.tile([64, 128], F32, tag="pc")
                    ps = stg_ps.tile([64, 128], F32, tag="ps")
                    nc.tensor.transpose(pc[:, :], fc[:, :], id_f32[:, :])
                    nc.tensor.transpose(ps[:, :], fs[:, :], id_f32[:, :])
                    sl = slice(st * 128, (st + 1) * 128)
                    nc.scalar.mul(tabqC[0:64, sl], pc[:, :], SCALE)
                    nc.scalar.mul(tabqC[64:128, sl], pc[:, :], SCALE)
                    nc.scalar.mul(tabqS[0:64, sl], ps[:, :], SCALE)
                    nc.scalar.mul(tabqS[64:128, sl], ps[:, :], SCALE)
                    nc.vector.tensor_copy(tabkC[0:64, sl], pc[:, :])
                    nc.vector.tensor_copy(tabkC[64:128, sl], pc[:, :])
                    nc.vector.tensor_copy(tabkS[0:64, sl], ps[:, :])
                    nc.vector.tensor_copy(tabkS[64:128, sl], ps[:, :])

                # lora downs: stacked [48, D] -> bf16 -> padded stationaries
                # (each 16-row group at a 32-aligned psum partition base)
                ldf = stage.tile([48, D], F32, tag="big_f32")
                nc.sync.dma_start(out=ldf[0:16, :], in_=g["lqd_ext"][:, :])
                nc.sync.dma_start(out=ldf[16:32, :], in_=g["lkd_ext"][:, :])
                nc.sync.dma_start(out=ldf[32:48, :], in_=g["lvd_ext"][:, :])
                ldh = stage.tile([48, D], BF16, tag="big_bf16")
                nc.vector.tensor_copy(ldh[:, :], ldf[:, :])
                for dt in range(NDT):
                    pt_ = stg_ps.tile([128, 48], BF16, tag="pldt")
                    nc.tensor.transpose(
                        pt_[:, :], ldh[:, dt * 128 : (dt + 1) * 128], id_bf16[0:48, 0:48]
                    )
                    base = dt * 80
                    nc.scalar.copy(ldsT[:, base : base + 16], pt_[:, 0:16])
                    nc.scalar.copy(ldsT[:, base + 32 : base + 48], pt_[:, 16:32])
                    nc.scalar.copy(ldsT[:, base + 64 : base + 80], pt_[:, 32:48])

                # lora ups: rows deinterleaved for q/k, natural for v
                for name, ext in (("q", g["lqu_ext"]), ("k", g["lku_ext"]), ("v", g["lvu_ext"])):
                    rr = ext.rearrange("(mt q p) r -> p mt q r", mt=NMT, q=64, p=2)
                    for mt in range(NMT):
                        uf = stage.tile([128, R], F32, tag="uf")
                        if name in ("q", "k"):
                            nc.sync.dma_start(out=uf[0:64, :], in_=rr[0, mt])
                            nc.sync.dma_start(out=uf[64:128, :], in_=rr[1, mt])
                        else:
                            nc.sync.dma_start(
                                out=uf[:, :], in_=ext[mt * 128 : (mt + 1) * 128, :]
                            )
                        uh = stage.tile([128, R], BF16, tag="uh")
                        nc.vector.tensor_copy(uh[:, :], uf[:, :])
                        pu = stg_ps.tile([R, 128], BF16, tag="pu")
                        nc.tensor.transpose(pu[:, :], uh[:, :], id_bf16[:, :])
                        nc.scalar.copy(upTs[name][:, mt * 128 : (mt + 1) * 128], pu[:, :])


            # --------------------------------------------------------------
            # Phase B1: lora mids + V projection (SBUF-resident V)
            # --------------------------------------------------------------
            with tc.tile_pool(name="wtv", bufs=16) as wtv_pool, \
                 tc.tile_pool(name="proj_ps1", bufs=2, space="PSUM") as proj_ps1, \
                 tc.tile_pool(name="tw_ps", bufs=2, space="PSUM") as tw_ps, \
                 tc.tile_pool(name="ld_ps", bufs=1, space="PSUM") as ld_ps:
                for qc in range(NQC):
                    pld = ld_ps.tile([80, SC], F32, tag="pld")
                    for dt in range(NDT):
                        nc.tensor.matmul(
                            pld[:, :],
                            ldsT[:, dt * 80 : (dt + 1) * 80],
                            XT[dt][:, qc * SC : (qc + 1) * SC],
                            start=(dt == 0),
                            stop=(dt == NDT - 1),
                        )
                    sl = slice(qc * SC, (qc + 1) * SC)
                    nc.scalar.mul(ldTs["q"][:, sl], pld[0:16, :], LORA_SCALE)
                    nc.scalar.mul(ldTs["k"][:, sl], pld[32:48, :], LORA_SCALE)
                    nc.scalar.mul(ldTs["v"][:, sl], pld[64:80, :], LORA_SCALE)

                wts = [
                    wtv_pool.tile([128, M], BF16, tag="wtv", name=f"wvT{dt}")
                    for dt in range(NDT)
                ]
                for mt in range(NMT):
                    wh = stage.tile([128, D], BF16, tag="big_bf16")
                    nc.gpsimd.dma_start(
                        out=wh[:, :], in_=g["wv_ext"][mt * 128 : (mt + 1) * 128, :]
                    )
                    for dt in range(NDT):
                        ptw = tw_ps.tile([128, 128], BF16, tag="ptw")
                        nc.tensor.transpose(
                            ptw[:, :], wh[:, dt * 128 : (dt + 1) * 128], id_bf16[:, :]
                        )
                        dst = wts[dt][:, mt * 128 : (mt + 1) * 128]
                        if dt % 2 == 0:
                            nc.scalar.copy(dst, ptw[:, :])
                        else:
                            nc.vector.tensor_copy(dst, ptw[:, :])
                for st in range(NST):
                    pv = proj_ps1.tile([128, M], F32, tag="pv")
                    ssl = slice(st * 128, (st + 1) * 128)
                    for sub in range(2):
                        psl = slice(sub * SC, (sub + 1) * SC)
                        for dt in range(NDT):
                            nc.tensor.matmul(
                                pv[:, psl], XT[dt][:, ssl], wts[dt][:, psl],
                                start=(dt == 0), stop=False,
                            )
                        nc.tensor.matmul(
                            pv[:, psl], ldTs["v"][:, ssl], upTs["v"][:, psl],
                            start=False, stop=True,
                        )
                    nc.vector.tensor_copy(VS[st][:, :], pv[:, :])

            # --------------------------------------------------------------
            # Phase A2: wq/wk/wo casts (no PSUM; overlaps B1 via scheduler)
            # --------------------------------------------------------------
            for oc in range(NQC):
                for half in range(4):
                    wh = stage.tile([128, M], BF16, tag="big_bf16")
                    r0 = oc * SC + half * 128
                    nc.gpsimd.dma_start(out=wh[:, 0:M], in_=wo_ext[r0 : r0 + 128, :])
                    nc.sync.dma_start(
                        out=wob[oc][half * 128 : (half + 1) * 128, :], in_=wh[:, 0:M]
                    )

            # --------------------------------------------------------------
            # Phase B2: Q^T / K^T, head outer (q then k per head so that
            # attention on head h can begin while head h+1 projects), with
            # the rope eviction split across ACT (psum spill), DVE, and
            # GpSimd (2 of the 4 products).
            # --------------------------------------------------------------
            HS = S // 2  # 1024: half the sequence per psum tile
            with tc.tile_pool(name="wtt", bufs=20) as wtt_pool, \
                 tc.tile_pool(name="proj_ps2", bufs=2, space="PSUM") as proj_ps2, \
                 tc.tile_pool(name="tw2_ps", bufs=2, space="PSUM") as tw2_ps, \
                 tc.tile_pool(name="rope_tmp", bufs=3) as rope_tmp, \
                 tc.tile_pool(name="qk_ev", bufs=6) as qk_ev:
                rr_q = g["wq_ext"].rearrange("(mt q p) d -> p mt q d", mt=NMT, q=64, p=2)
                rr_k = g["wk_ext"].rearrange("(mt q p) d -> p mt q d", mt=NMT, q=64, p=2)
                for mt in range(NMT):
                    for which, rr, outd, tabC, tabS in (
                        ("q", rr_q, qtd, tabqC, tabqS),
                        ("k", rr_k, ktd, tabkC, tabkS),
                    ):
                        wh = stage.tile([128, D], BF16, tag="big_bf16")
                        nc.gpsimd.dma_start(out=wh[0:64, :], in_=rr[0, mt])
                        nc.gpsimd.dma_start(out=wh[64:128, :], in_=rr[1, mt])
                        wts = []
                        for dt in range(NDT):
                            ptw = tw2_ps.tile([128, 128], BF16, tag="ptw2")
                            nc.tensor.transpose(
                                ptw[:, :], wh[:, dt * 128 : (dt + 1) * 128], id_bf16[:, :]
                            )
                            wtt = wtt_pool.tile([128, 128], BF16, tag="wtt")
                            nc.scalar.copy(wtt[:, :], ptw[:, :])
                            wts.append(wtt)
                        for half in range(2):
                            sl = slice(half * HS, (half + 1) * HS)
                            pq = proj_ps2.tile([128, HS], F32, tag="pq")
                            for sub in range(2):
                                psl = slice(sub * SC, (sub + 1) * SC)
                                xsl = slice(half * HS + sub * SC, half * HS + (sub + 1) * SC)
                                for dt in range(NDT):
                                    nc.tensor.matmul(
                                        pq[:, psl], wts[dt][:, :], XT[dt][:, xsl],
                                        start=(dt == 0), stop=False,
                                    )
                                nc.tensor.matmul(
                                    pq[:, psl],
                                    upTs[which][:, mt * 128 : (mt + 1) * 128],
                                    ldTs[which][:, xsl],
                                    start=False, stop=True,
                                )
                            # rope on deinterleaved pairs: rows 0:64 = a
                            # (even lanes), 64:128 = b (odd lanes); tab rows
                            # 0:64 cos, 64:128 sin.
                            if mt < 6:
                                qsb = rope_tmp.tile([128, HS], BF16, tag="qsb")
                                nc.scalar.copy(qsb[:, :], pq[:, :])
                            t1 = rope_tmp.tile([128, HS], BF16, tag="t1")
                            t2 = rope_tmp.tile([128, HS], BF16, tag="t2")
                            # one full-width product, fully aligned:
                            # t1 rows 0:64 = a*cos, rows 64:128 = b*cos.
                            # GpSimd helps mid-phase; near the phase tail its
                            # queue backlog would delay the final evictions
                            # (and the PSUM release attention waits on), so
                            # the last heads use DVE instead.
                            if mt < 6:
                                nc.gpsimd.tensor_mul(t1[:, :], qsb[:, :], tabC[:, sl])
                            else:
                                nc.vector.tensor_mul(t1[:, :], pq[:, :], tabC[:, sl])
                            # cross-half products from psum on DVE (the psum
                            # operand may change base partition):
                            # t2 rows 0:64 = b*sin, rows 64:128 = a*sin
                            nc.vector.tensor_mul(t2[0:64, :], pq[64:128, :], tabS[0:64, sl])
                            nc.vector.tensor_mul(t2[64:128, :], pq[0:64, :], tabS[64:128, sl])
                            ev = qk_ev.tile([128, HS], BF16, tag="ev")
                            # even out = a*cos - b*sin ; odd out = a*sin + b*cos
                            nc.vector.tensor_sub(ev[0:64, :], t1[0:64, :], t2[0:64, :])
                            nc.vector.tensor_add(ev[64:128, :], t2[64:128, :], t1[64:128, :])
                            nc.sync.dma_start(out=outd[mt][:, sl], in_=ev[:, :])

    # ----------------------------------------------------------------------
    # Phases C+D interleaved, query-chunk outer: attention for all heads of
    # one 512-query chunk, then that chunk's o_proj rows and its pairwise
    # ReduceScatter — the collective for chunk c overlaps attention of
    # chunk c+1.
    # ----------------------------------------------------------------------
    with tc.tile_pool(name="at", bufs=2) as at_pool, \
         tc.tile_pool(name="wo_t", bufs=32) as wo_pool, \
         tc.tile_pool(name="qk_ld", bufs=6) as qk_ld, \
         tc.tile_pool(name="pt", bufs=8) as pt_pool, \
         tc.tile_pool(name="o_sb", bufs=4) as o_sb, \
         tc.tile_pool(name="sc_ps", bufs=4, space="PSUM") as sc_ps, \
         tc.tile_pool(name="pv_ps", bufs=2, space="PSUM") as pv_ps, \
         tc.tile_pool(name="l_ps", bufs=2, space="PSUM") as l_ps, \
         tc.tile_pool(name="linv", bufs=4) as linv_pool:

        wts_o = {}

        pre = {}
        for qc in range(NQC):
            nk = 4 * (qc + 1)
            AT = {}
            pending = []
            for h in range(H_LOC):
                if h in pre:
                    kth, qth = pre.pop(h)
                else:
                    kth = qk_ld.tile([128, S], BF16, tag="kth")
                    nc.sync.dma_start(
                        out=kth[:, 0 : nk * 128], in_=g["ktd"][h][:, 0 : nk * 128]
                    )
                    qth = qk_ld.tile([128, SC], BF16, tag="qth")
                    nc.sync.dma_start(
                        out=qth[:, :], in_=g["qtd"][h][:, qc * SC : (qc + 1) * SC]
                    )
                po = pv_ps.tile([128, SC], F32, tag="po")
                pl = l_ps.tile([1, SC], F32, tag="pl")
                for ki in range(nk):
                    q_off = max(0, ki * 128 - qc * SC)
                    csl = slice(q_off, SC)
                    ps_ = sc_ps.tile([128, SC], F32, tag="ps")
                    diag = ki * 128 >= qc * SC
                    nc.tensor.matmul(
                        ps_[:, csl],
                        kth[:, ki * 128 : (ki + 1) * 128],
                        qth[:, csl],
                        start=True, stop=not diag,
                    )
                    if diag:  # causal mask accumulated on the PE itself
                        nc.tensor.matmul(
                            ps_[:, q_off : q_off + 128],
                            id_bf16[:, :],
                            triT_h[:, :],
                            start=False, stop=True,
                        )
                    pt_ = pt_pool.tile([128, SC], BF16, tag="pt")
                    nc.scalar.activation(pt_[:, csl], ps_[:, csl], AF.Exp)
                    nc.tensor.matmul(
                        pl[:, csl], ones_col[:, :], pt_[:, csl],
                        start=(ki == 0), stop=(ki == nk - 1),
                    )
                    nc.tensor.matmul(
                        po[:, csl],
                        VS[ki][:, h * 128 : (h + 1) * 128],
                        pt_[:, csl],
                        start=(ki == 0), stop=(ki == nk - 1),
                    )
                # head epilogue is emitted one head late (see below) so the
                # broadcast matmul never stalls the in-order PE queue on the
                # reciprocal.
                pending.append((h, po, pl))
                if len(pending) > 1:
                    _attn_epilogue(nc, g, sc_ps, linv_pool, at_pool, AT, qc, *pending.pop(0))

            while pending:
                _attn_epilogue(nc, g, sc_ps, linv_pool, at_pool, AT, qc, *pending.pop(0))

            if qc == 0:
                # o_proj weights load here -- after chunk 0's K/Q streams,
                # before they are first consumed (their 32 slow xbar
                # transposes would otherwise delay attention's first loads)
                for oc in range(NQC):
                    for mt in range(NMT):
                        wtt = wo_pool.tile([128, SC], BF16, tag="wot")
                        nc.sync.dma_start_transpose(
                            out=wtt[:, :], in_=wob[oc][:, mt * 128 : (mt + 1) * 128]
                        )
                        wts_o[(oc, mt)] = wtt

            # prefetch the next chunk's first heads before o_proj floods the
            # DMA queues with output traffic
            if qc < NQC - 1:
                nk2 = 4 * (qc + 2)
                # later chunks stream more K data and face more RS traffic:
                # prefetch deeper for them
                for h2 in range(2 + qc):
                    kth2 = qk_ld.tile([128, S], BF16, tag="kth", name=f"kpre{qc}_{h2}")
                    nc.sync.dma_start(
                        out=kth2[:, 0 : nk2 * 128], in_=g["ktd"][h2][:, 0 : nk2 * 128]
                    )
                    qth2 = qk_ld.tile([128, SC], BF16, tag="qth", name=f"qpre{qc}_{h2}")
                    nc.sync.dma_start(
                        out=qth2[:, :],
                        in_=g["qtd"][h2][:, (qc + 1) * SC : (qc + 2) * SC],
                    )
                    pre[h2] = (kth2, qth2)

            # o_proj rows of this chunk; ReduceScatter per 128-row st tile so
            # the collective for st overlaps o_proj of st+1 (and the tail
            # exposes only the last 128-row collective)
            for st in range(qc * 4, (qc + 1) * 4):
                for oc in range(NQC):
                    posum = sc_ps.tile([128, SC], F32, tag="ps", name=f"posum{st}_{oc}")
                    for mt in range(NMT):
                        nc.tensor.matmul(
                            posum[:, :],
                            AT[mt][:, (st % 4) * 128 : (st % 4 + 1) * 128],
                            wts_o[(oc, mt)][:, :],
                            start=(mt == 0), stop=(mt == NMT - 1),
                        )
                    ot = o_sb.tile([128, SC], F32, tag="ot")
                    nc.vector.tensor_copy(ot[:, :], posum[:, :])
                    nc.sync.dma_start(
                        out=out_partial[
                            st * 128 : (st + 1) * 128, oc * SC : (oc + 1) * SC
                        ],
                        in_=ot[:, :],
                    )
                r0, r1 = st * 128, (st + 1) * 128
                nc.gpsimd.collective_compute(
                    "ReduceScatter",
                    mybir.AluOpType.add,
                    replica_groups=[[0, 1], [2, 3], [4, 5], [6, 7]],
                    ins=[out_partial[r0:r1, :].opt()],
                    outs=[out_rs[r0 // 2 : r1 // 2, :].opt()],
                )
                nc.sync.dma_start(
                    out=out_ext[r0 // 2 : r1 // 2, :],
                    in_=out_rs[r0 // 2 : r1 // 2, :],
                )

    vs_ctx.__exit__(None, None, None)

# ---------------------------------------------------------------------------
# Host entry point
# ---------------------------------------------------------------------------

_NC_CACHE = None


def _get_nc():
    global _NC_CACHE
    if _NC_CACHE is None:
        _NC_CACHE = build_nc()
    return _NC_CACHE


def kernel(
    x, wq, wk, wv, wo,
    lq_down, lq_up, lk_down, lk_up, lv_down, lv_up,
    freqs_cos, freqs_sin, mask,
):
    """Full inputs in, full [B, S, D] output out; 8-core SPMD inside."""
    from concourse.bass_utils import run_bass_kernel_spmd

    x = np.asarray(x, dtype=np.float32)
    maskc = np.ascontiguousarray(np.asarray(mask, dtype=np.float32)[:128, :128])
    fcos = np.ascontiguousarray(np.asarray(freqs_cos, dtype=np.float32))
    fsin = np.ascontiguousarray(np.asarray(freqs_sin, dtype=np.float32))

    in_maps = []
    for c in range(N_CORES):
        b, j = c // 2, c % 2
        msl = slice(j * M, (j + 1) * M)
        in_maps.append({
            "x": np.ascontiguousarray(x[b]),
            "wq": np.ascontiguousarray(np.asarray(wq, np.float32)[msl, :]),
            "wk": np.ascontiguousarray(np.asarray(wk, np.float32)[msl, :]),
            "wv": np.ascontiguousarray(np.asarray(wv, np.float32)[msl, :]),
            "wo": np.ascontiguousarray(np.asarray(wo, np.float32)[:, msl]),
            "lqd": np.ascontiguousarray(np.asarray(lq_down, np.float32)),
            "lkd": np.ascontiguousarray(np.asarray(lk_down, np.float32)),
            "lvd": np.ascontiguousarray(np.asarray(lv_down, np.float32)),
            "lqu": np.ascontiguousarray(np.asarray(lq_up, np.float32)[msl, :]),
            "lku": np.ascontiguousarray(np.asarray(lk_up, np.float32)[msl, :]),
            "lvu": np.ascontiguousarray(np.asarray(lv_up, np.float32)[msl, :]),
            "fcos": fcos,
            "fsin": fsin,
            "maskc": maskc,
        })

    nc = _get_nc()
    res = run_bass_kernel_spmd(nc, in_maps, list(range(N_CORES)), trace=False)

    return assemble(res.results)


def assemble(results):
    # Per-st-tile pairwise ReduceScatter: core (2b+j) holds, for every st
    # tile, the reduced global rows [st*128 + j*64, st*128 + (j+1)*64) of
    # batch b at local rows [st*64, (st+1)*64).
    out = np.empty((B, S, D), dtype=np.float32)
    for b in range(B):
        for j in range(2):
            buf = results[2 * b + j]["out"]
            for st in range(NST):
                out[b, st * 128 + j * 64 : st * 128 + (j + 1) * 64, :] = \
                    buf[st * 64 : (st + 1) * 64, :]
    return out

